# revision 14
# baseline (speedup 1.0000x reference)
"""Bass/Trainium2 kernel for nn_Attention_75007308857927.

Reference computation (B=4, S=2048, D=1024):
    Q = X @ Wq.T ; K = X @ Wk.T ; V = X @ Wv.T         (per batch)
    Qn, Kn = row-normalized Q, K
    scores = (Qn @ Kn.T) * m      m[i,j] = 1 if (j > i) or masks[j]==0 else 0
    out = scores @ V

NOTE the mask orientation: m = 1 KEEPS a position (j > i, i.e. future,
or padded key j). Score blocks fully above the diagonal therefore need
no mask; blocks fully below need only a per-key padding column; only the
8 near-diagonal slots per 512-query block need a full mask tile.

Sharding: 8 cores = 4 batches x 2 query-tile sets, query tiles
interleaved (core h owns global tiles {2g+h}) so the near-diagonal
block set is core-invariant and one SPMD program serves both cores;
per-core mask data handles the rest. Keys stay with their owner,
AllGathered in slot order [rank0 tiles (even), rank1 tiles (odd)].

Device algebra per core (matmuls contract over the partition dim):
    KT[e,j']  = sum_d WkT[d,e] * XQ[d,j']        (own keys j')
    kinv[j']  = rsqrt(sum_e KT[e,j']^2)
    V'[j',e]  = (sum_d XQ[d,j'] WvT[d,e]) * kinv[j']
    KT, V'    = AllGather over the core pair     (slot order)
    QT[e,i]   = sum_d WqT[d,e] * XQ[d,i]
    qinv[i]   = rsqrt(sum_e QT[e,i]^2)
    ST[j,i]   = (sum_e KT[e,j] QT[e,i]) * maskT[j,i]   (skipped above diag;
                pad-only column mask off-diagonal, full mask near diagonal)
    out[i,d]  = (sum_{j in slots(g)} ST[j,i] V'[j,d]) * qinv[i]

Weights and XQ are host-pretiled into SBUF image order so the first
projection chain can start after ~1.25MB of DMA.

bf16 matmul operands, f32 PSUM accumulation. Precision vs f32 reference:
absmax error ~0.4% of output scale.
"""

import numpy as np
import ml_dtypes

B, S, D = 4, 2048, 1024
HALF = S // 2  # queries/keys per core
N_CORES = 8
P = 128
DC = D // P    # 8 contraction chunks over d
ET = D // P    # 8 e-tiles
JT = S // P    # 16 key slots (global)
JTH = HALF // P  # 8 own tiles
I5 = HALF // 512  # 2

# query-tile ownership: interleaved (balances causal work, core-invariant
# union skip pattern). Core h owns global tiles 2g+h, g=0..7.
TILES = [[2 * g for g in range(JTH)], [2 * g + 1 for g in range(JTH)]]
# SBUF key slot s holds global key tile KEYTILE[s] (rank0 evens, rank1 odds)
KEYTILE = TILES[0] + TILES[1]

BF16 = ml_dtypes.bfloat16

_CACHE = {}


def _fkind(ib, s):
    """Classification of score block (ib, slot s), core-invariant.

    The reference mask KEEPS masked positions: m = (j > i) | pad_j.
    Blocks fully above the diagonal (all j > i) need no mask at all;
    blocks fully below need only the per-key padding column; the 8
    near-diagonal slots need a full per-core mask tile.
    """
    l = s % 8
    if l > 4 * ib + 3:
        return 'ones'
    return 'diag' if l >= 4 * ib else 'pad'


def _emit(ctx, tc, xq, wkt, wvt, wqt, maskd, padc, out,
          kt_own, kt_gath, v_own, v_gath):
    from concourse import mybir

    nc = tc.nc
    dtb = mybir.dt.bfloat16
    dtf = mybir.dt.float32
    dt8 = mybir.dt.float8e4

    # ---- SBUF pools -------------------------------------------------------
    xq_p = ctx.enter_context(tc.tile_pool(name="xq", bufs=1))
    # weights + scores blocks share one pool: all tiles are 16KB/partition
    w_p = ctx.enter_context(tc.tile_pool(name="wst", bufs=3))
    kt_p = ctx.enter_context(tc.tile_pool(name="kt", bufs=1))
    qt_p = ctx.enter_context(tc.tile_pool(name="qt", bufs=1))
    vp_p = ctx.enter_context(tc.tile_pool(name="vp", bufs=1))
    row_p = ctx.enter_context(tc.tile_pool(name="rows", bufs=1))
    sq_p = ctx.enter_context(tc.tile_pool(name="sq", bufs=3))
    stg_p = ctx.enter_context(tc.tile_pool(name="stg", bufs=10))
    mk_p = ctx.enter_context(tc.tile_pool(name="mk", bufs=4))
    ev_p = ctx.enter_context(tc.tile_pool(name="ev", bufs=3))
    ps_p = ctx.enter_context(tc.tile_pool(name="psmm", bufs=5, space="PSUM"))
    psr_p = ctx.enter_context(tc.tile_pool(name="psrow", bufs=2, space="PSUM"))
    psc_p = ctx.enter_context(tc.tile_pool(name="pscol", bufs=1, space="PSUM"))

    xq_s = xq_p.tile([P, I5 * DC * 512], dtb, tag="xq")  # [(j5 dc) tiled]
    wkt_s = w_p.tile([P, ET * DC * P], dtb, tag="w")     # [(et dc) tiled]
    wvt_s = w_p.tile([P, DC * D], dtb, tag="w")          # [d%128, dc*1024+e]
    wqt_s = w_p.tile([P, DC * D], dtb, tag="w")
    kt_s = kt_p.tile([P, ET * S], dtb, tag="kt")         # [e%128, et*2048+j]
    qt_s = qt_p.tile([P, ET * HALF], dtb, tag="qt")      # [e%128, et*1024+i]
    vp_s = vp_p.tile([P, JT * D], dtb, tag="vp")         # [j%128, s*1024+d]

    ones_b = row_p.tile([P, 1], dtb, tag="ones_b")
    ones_f = row_p.tile([1, 1], dtf, tag="ones_f")
    ksq_row = row_p.tile([1, HALF], dtf, tag="sqrow")
    qsq_row = row_p.tile([1, HALF], dtf, tag="sqrow")
    ksq_col = row_p.tile([P, JTH], dtf, tag="ksqc")    # col c <-> own j-chunk c
    krec_col = row_p.tile([P, JTH], dtf, tag="krecc")
    kinv_col = row_p.tile([P, JTH], dtf, tag="kinvc")
    qsq_col = row_p.tile([P, ET], dtf, tag="qsqc")
    qrec_col = row_p.tile([P, ET], dtf, tag="qrecc")
    qinv_col = row_p.tile([P, ET], dtf, tag="qinvc")
    pad_s = row_p.tile([P, JT], dtf, tag="pad")        # key padding per slot

    nc.vector.memset(ones_b[:], 1.0)
    nc.vector.memset(ones_f[:], 1.0)

    # ---- input DMAs, in consumption order across two HW queues -----------
    # B(j5=0,et=0) needs wkt tiles (et=0,*) + xq tiles (j5=0,*): the first
    # chunks are split across the sync and scalar queues so the first
    # projection chain can start ~3us in. Weights stream in behind on
    # scalar; the sync queue frees up early for kt_own/v_own stores.
    def xq_dma(q, j5, dclist):
        for dc in dclist:
            o = (j5 * DC + dc) * 512
            q.dma_start(xq_s[:, o:o + 512], xq[:, o:o + 512])
    nc.sync.dma_start(wkt_s[:, 0:DC * P], wkt[:, 0:DC * P])
    xq_dma(nc.scalar, 0, range(0, 4))
    xq_dma(nc.sync, 0, range(4, DC))
    for et in range(1, ET):
        nc.sync.dma_start(wkt_s[:, et * DC * P:(et + 1) * DC * P],
                          wkt[:, et * DC * P:(et + 1) * DC * P])
    # sync is now free for the kt_own/v_own stores feeding the AllGathers.
    xq_dma(nc.scalar, 1, range(0, DC))
    nc.scalar.dma_start(pad_s[:], padc[:, :])
    # wvt/wqt ride the otherwise-idle gpsimd queue (first gather load
    # only needs it after the first AllGather completes)
    for w_s, w_d in ((wvt_s, wvt), (wqt_s, wqt)):
        for dc in range(DC):
            nc.gpsimd.dma_start(w_s[:, dc * D:(dc + 1) * D],
                                w_d[dc * P:(dc + 1) * P, :])

    groups = [[0, 1], [2, 3], [4, 5], [6, 7]]

    def xq_rhs(j5, dc):      # [128, 512] own queries j5*512.. for d-chunk dc
        o = (j5 * DC + dc) * 512
        return xq_s[:, o:o + 512]

    def xq_lhsT(jt, dc):     # [128, 128] own queries jt*128.. for d-chunk dc
        o = ((jt // 4) * DC + dc) * 512 + (jt % 4) * P
        return xq_s[:, o:o + P]

    # ---- phase B: KT for own keys + k sumsq ------------------------------
    for j5 in range(I5):
        ksq_ps = psr_p.tile([1, 512], dtf, tag="psrow")
        for et in range(ET):
            ps = ps_p.tile([P, 512], dtf, tag="psmm")
            for dc in range(DC):
                nc.tensor.matmul(
                    ps[:],
                    lhsT=wkt_s[:, (et * DC + dc) * P:(et * DC + dc + 1) * P],
                    rhs=xq_rhs(j5, dc),
                    start=(dc == 0), stop=(dc == DC - 1),
                )
            stg = stg_p.tile([P, 512], dtb, tag="stg")
            nc.vector.tensor_copy(stg[:], ps[:])
            nc.sync.dma_start(kt_own[j5][et * P:(et + 1) * P, :], stg[:])
            sq = sq_p.tile([P, 512], dtb, tag="sq")
            nc.scalar.square(sq[:], ps[:])
            nc.tensor.matmul(ksq_ps[:], lhsT=ones_b[:], rhs=sq[:],
                             start=(et == 0), stop=(et == ET - 1))
        # gather this half of KT as soon as its stores land
        nc.gpsimd.collective_compute(
            "AllGather", mybir.AluOpType.bypass, replica_groups=groups,
            ins=[kt_own[j5][:]], outs=[kt_gath[j5][:]])
        nc.vector.tensor_copy(ksq_row[0:1, j5 * 512: j5 * 512 + 512], ksq_ps[:])
        for cc in range(4):
            c = j5 * 4 + cc
            pc = psc_p.tile([P, 1], dtf, tag="pscol")
            nc.tensor.matmul(pc[:], lhsT=ksq_row[0:1, c * P:(c + 1) * P],
                             rhs=ones_f[:], start=True, stop=True)
            nc.vector.tensor_copy(ksq_col[:, c:c + 1], pc[:])

    # kinv for own keys, 128-way parallel in column layout
    nc.vector.reciprocal(krec_col[:], ksq_col[:])
    nc.scalar.sqrt(kinv_col[:], krec_col[:])

    # load gathered KT halves to SBUF (slot-order j: rank r half, j5 piece)
    kt3 = kt_s[:].rearrange("p (et j) -> p et j", et=ET, j=S)
    for j5 in range(I5):
        for r in range(2):
            src3 = kt_gath[j5][r].rearrange("(et p) j -> p et j", p=P)
            for eg in range(0, ET, 4):
                nc.gpsimd.dma_start(
                    kt3[:, eg:eg + 4,
                        r * HALF + j5 * 512: r * HALF + j5 * 512 + 512],
                    src3[:, eg:eg + 4, :])

    # ---- phase D: V' = V * kinv[j] for own keys --------------------------
    # e5 pair shares the stationary xq tile per dc step (weight reuse)
    for jt in range(JTH):
        ps_a = ps_p.tile([P, 512], dtf, tag="psmm")
        ps_b = ps_p.tile([P, 512], dtf, tag="psmm")
        pspair = [ps_a, ps_b]
        for dc in range(DC):
            for e5 in range(2):
                nc.tensor.matmul(
                    pspair[e5][:],
                    lhsT=xq_lhsT(jt, dc),
                    rhs=wvt_s[:, dc * D + e5 * 512: dc * D + e5 * 512 + 512],
                    start=(dc == 0), stop=(dc == DC - 1),
                )
        for e5 in range(2):
            stg = stg_p.tile([P, 512], dtb, tag="stg")
            nc.vector.tensor_scalar_mul(stg[:], pspair[e5][:], kinv_col[:, jt:jt + 1])
            nc.sync.dma_start(
                v_own[jt // 4][(jt % 4) * P:(jt % 4 + 1) * P,
                               e5 * 512: e5 * 512 + 512], stg[:])
        if jt % 4 == 3:
            nc.gpsimd.collective_compute(
                "AllGather", mybir.AluOpType.bypass, replica_groups=groups,
                ins=[v_own[jt // 4][:]], outs=[v_gath[jt // 4][:]])

    for half in range(2):
        for r in range(2):
            o = (r * JTH + half * 4) * D
            dst = vp_s[:, o: o + 4 * D]
            dst = dst.rearrange("p (jtl e) -> p jtl e", jtl=4, e=D)
            src_ap = v_gath[half][r].rearrange("(jtl p) e -> p jtl e", p=P)
            nc.gpsimd.dma_start(dst[:, :, :], src_ap[:, :, :])

    # ---- phase E: QT + q sumsq -------------------------------------------
    for i5 in range(I5):
        qsq_ps = psr_p.tile([1, 512], dtf, tag="psrow")
        for et in range(ET):
            ps = ps_p.tile([P, 512], dtf, tag="psmm")
            for dc in range(DC):
                nc.tensor.matmul(
                    ps[:],
                    lhsT=wqt_s[:, dc * D + et * P: dc * D + (et + 1) * P],
                    rhs=xq_rhs(i5, dc),
                    start=(dc == 0), stop=(dc == DC - 1),
                )
            qtsl = qt_s[:, et * HALF + i5 * 512: et * HALF + i5 * 512 + 512]
            nc.vector.tensor_copy(qtsl, ps[:])
            sq = sq_p.tile([P, 512], dtb, tag="sq")
            nc.scalar.square(sq[:], ps[:])
            nc.tensor.matmul(qsq_ps[:], lhsT=ones_b[:], rhs=sq[:],
                             start=(et == 0), stop=(et == ET - 1))
        nc.vector.tensor_copy(qsq_row[0:1, i5 * 512: i5 * 512 + 512], qsq_ps[:])

    # ---- phase F: score blocks (union causal skip), then ------------------
    # ---- phase G: out blocks ---------------------------------------------
    st_blks = []
    for ib in range(I5):
        st_blk = w_p.tile([P, JT * 512], dtb, tag="w")  # [j%128, s*512+i]
        st_blks.append(st_blk)
        ndiag = 0
        for s in range(JT):
            kind = _fkind(ib, s)
            ps = ps_p.tile([P, 512], dtf, tag="psmm")
            for et in range(ET):
                nc.tensor.matmul(
                    ps[:],
                    lhsT=kt_s[:, et * S + s * P: et * S + (s + 1) * P],
                    rhs=qt_s[:, et * HALF + ib * 512: et * HALF + ib * 512 + 512],
                    start=(et == 0), stop=(et == ET - 1),
                )
            dst = st_blk[:, s * 512:(s + 1) * 512]
            if kind == 'diag':
                mk = mk_p.tile([P, 512], dt8, tag="mk")
                nc.scalar.dma_start(mk[:], maskd[(ib * 8 + ndiag) * P:
                                               (ib * 8 + ndiag + 1) * P, :])
                ndiag += 1
                nc.vector.tensor_mul(dst, ps[:], mk[:])
            elif kind == 'pad':
                nc.vector.tensor_scalar_mul(dst, ps[:], pad_s[:, s:s + 1])
            else:  # 'ones': fully above the diagonal, mask is all-ones
                nc.vector.tensor_copy(dst, ps[:])
    # q-norm chain (deferred so F's matmuls aren't blocked behind it)
    for c in range(ET):
        pc = psc_p.tile([P, 1], dtf, tag="pscol")
        nc.tensor.matmul(pc[:], lhsT=qsq_row[0:1, c * P:(c + 1) * P],
                         rhs=ones_f[:], start=True, stop=True)
        nc.vector.tensor_copy(qsq_col[:, c:c + 1], pc[:])
    nc.vector.reciprocal(qrec_col[:], qsq_col[:])
    nc.scalar.sqrt(qinv_col[:], qrec_col[:])

    for ib in range(I5):
        st_blk = st_blks[ib]
        for itl in range(4):
            g = ib * 4 + itl          # local i-tile
            for d5 in range(2):
                ps = ps_p.tile([P, 512], dtf, tag="psmm")
                for s in range(JT):
                    nc.tensor.matmul(
                        ps[:],
                        lhsT=st_blk[:, s * 512 + itl * P: s * 512 + (itl + 1) * P],
                        rhs=vp_s[:, s * D + d5 * 512: s * D + d5 * 512 + 512],
                        start=(s == 0), stop=(s == JT - 1),
                    )
                ot = ev_p.tile([P, 512], dtf, tag="ev")
                nc.vector.tensor_scalar_mul(ot[:], ps[:], qinv_col[:, g:g + 1])
                nc.sync.dma_start(out[g * P:(g + 1) * P, d5 * 512: d5 * 512 + 512],
                                  ot[:])


def _build():
    if "nc" in _CACHE:
        return _CACHE["nc"]
    import concourse.tile as tile
    from concourse import bacc, mybir

    dtb = mybir.dt.bfloat16
    dtf = mybir.dt.float32
    nc = bacc.Bacc("TRN2", target_bir_lowering=False, debug=False,
                   enable_asserts=True, num_devices=N_CORES)
    xq = nc.dram_tensor("xq", [P, I5 * DC * 512], dtb, kind="ExternalInput").ap()
    wkt = nc.dram_tensor("wkt", [P, ET * DC * P], dtb, kind="ExternalInput").ap()
    wvt = nc.dram_tensor("wvt", [D, D], dtb, kind="ExternalInput").ap()
    wqt = nc.dram_tensor("wqt", [D, D], dtb, kind="ExternalInput").ap()
    maskd = nc.dram_tensor("maskd", [I5 * 8 * P, 512],
                           mybir.dt.float8e4,
                           kind="ExternalInput").ap()
    padc = nc.dram_tensor("padc", [P, JT], dtf, kind="ExternalInput").ap()
    out = nc.dram_tensor("out", [HALF, D], dtf, kind="ExternalOutput").ap()
    kt_own = [nc.dram_tensor(f"kt_own{i}", [D, 512], dtb).ap() for i in range(2)]
    kt_gath = [nc.dram_tensor(f"kt_gath{i}", [2, D, 512], dtb).ap()
               for i in range(2)]
    v_own = [nc.dram_tensor(f"v_own{i}", [512, D], dtb).ap() for i in range(2)]
    v_gath = [nc.dram_tensor(f"v_gath{i}", [2, 512, D], dtb).ap()
              for i in range(2)]

    from contextlib import ExitStack
    with tile.TileContext(nc) as tc:
        with ExitStack() as ctx:
            _emit(ctx, tc, xq, wkt, wvt, wqt, maskd, padc, out,
                  kt_own, kt_gath, v_own, v_gath)
    nc.compile()
    _CACHE["nc"] = nc
    return nc


def make_in_maps(X, masks, Wq, Wk, Wv):
    """Host-side sharding/layout: one input map per core."""
    in_maps = []
    wkt_h = np.ascontiguousarray(Wk.T).astype(BF16)   # [D, D] = [d, e]
    wvt_h = np.ascontiguousarray(Wv.T).astype(BF16)
    wqt_h = np.ascontiguousarray(Wq.T).astype(BF16)
    # wkt pretiled: img[p, (et*DC+dc)*128 + c] = wkt[dc*128+p, et*128+c]
    wkt_img = np.ascontiguousarray(
        wkt_h.reshape(DC, P, ET, P).transpose(1, 2, 0, 3).reshape(P, ET * DC * P))
    keycols = np.concatenate(
        [np.arange(t * P, (t + 1) * P) for t in KEYTILE])   # slot-order keys
    for c in range(N_CORES):
        b, h = c // 2, c % 2
        XT = X[b].T.astype(BF16)                            # [D, S]
        qcols = np.concatenate(
            [np.arange(t * P, (t + 1) * P) for t in TILES[h]])
        xo = XT[:, qcols]                                   # [D, 1024]
        # xq pretiled: img[p, (j5*DC+dc)*512 + c] = xo[dc*128+p, j5*512+c]
        xq_img = np.ascontiguousarray(
            xo.reshape(DC, P, I5, 512).transpose(1, 2, 0, 3)
            .reshape(P, I5 * DC * 512))
        padbit = (masks[b] == 0)                            # True = padded key
        # near-diagonal mask blocks, in (ib, slot) emission order
        mrows = []
        for ib in range(I5):
            icols = qcols[ib * 512:(ib + 1) * 512]
            for s in range(JT):
                if _fkind(ib, s) != 'diag':
                    continue
                jrows = keycols[s * P:(s + 1) * P]
                m = ((jrows[:, None] > icols[None, :]) |
                     padbit[jrows][:, None]).astype(BF16)
                mrows.append(m)
        maskd_h = np.ascontiguousarray(
            np.concatenate(mrows, axis=0).astype(ml_dtypes.float8_e4m3fn))
        padc = np.ascontiguousarray(
            padbit[keycols].astype(np.float32).reshape(JT, P).T)  # [128, JT]
        in_maps.append({
            "xq": xq_img,
            "wkt": wkt_img,
            "wvt": wvt_h,
            "wqt": wqt_h,
            "maskd": maskd_h,
            "padc": padc,
        })
    return in_maps


def run(in_maps, **kw):
    from concourse.bass_utils import run_bass_kernel_spmd
    nc = _build()
    return run_bass_kernel_spmd(nc, in_maps, list(range(N_CORES)), **kw)


def kernel(X, masks, Wq, Wk, Wv):
    X = np.asarray(X, dtype=np.float32)
    masks = np.asarray(masks)
    res = run(make_in_maps(X, masks, np.asarray(Wq, np.float32),
                           np.asarray(Wk, np.float32), np.asarray(Wv, np.float32)))
    out = np.empty((B, S, D), np.float32)
    for c in range(N_CORES):
        b, h = c // 2, c % 2
        for g, t in enumerate(TILES[h]):
            out[b, t * P:(t + 1) * P, :] = res.results[c]["out"][g * P:(g + 1) * P]
    return out


# revision 15
# speedup vs baseline: 1.0850x; 1.0850x over previous
"""Bass/Trainium2 kernel for nn_Attention_75007308857927.

Reference computation (B=4, S=2048, D=1024):
    Q = X @ Wq.T ; K = X @ Wk.T ; V = X @ Wv.T         (per batch)
    Qn, Kn = row-normalized Q, K
    scores = (Qn @ Kn.T) * m      m[i,j] = 1 if (j > i) or masks[j]==0 else 0
    out = scores @ V

NOTE the mask orientation: m = 1 KEEPS a position (j > i, i.e. future,
or padded key j). Score blocks fully above the diagonal therefore need
no mask; blocks fully below need only a per-key padding column; only the
8 near-diagonal slots per 512-query block need a full mask tile.

Sharding: 8 cores = 4 batches x 2 query-tile sets, query tiles
interleaved (core h owns global tiles {2g+h}) so the near-diagonal
block set is core-invariant and one SPMD program serves both cores;
per-core mask data handles the rest. Keys stay with their owner,
AllGathered in slot order [rank0 tiles (even), rank1 tiles (odd)].

Device algebra per core (matmuls contract over the partition dim):
    KT[e,j']  = sum_d WkT[d,e] * XQ[d,j']        (own keys j')
    kinv[j']  = rsqrt(sum_e KT[e,j']^2)
    V'[j',e]  = (sum_d XQ[d,j'] WvT[d,e]) * kinv[j']
    KT, V'    = AllGather over the core pair     (slot order)
    QT[e,i]   = sum_d WqT[d,e] * XQ[d,i]
    qinv[i]   = rsqrt(sum_e QT[e,i]^2)
    ST[j,i]   = (sum_e KT[e,j] QT[e,i]) * maskT[j,i]   (skipped above diag;
                pad-only column mask off-diagonal, full mask near diagonal)
    out[i,d]  = (sum_{j in slots(g)} ST[j,i] V'[j,d]) * qinv[i]

Weights and XQ are host-pretiled into SBUF image order so the first
projection chain can start after ~1.25MB of DMA.

bf16 matmul operands, f32 PSUM accumulation. Precision vs f32 reference:
absmax error ~0.4% of output scale.
"""

import numpy as np
import ml_dtypes

B, S, D = 4, 2048, 1024
HALF = S // 2  # queries/keys per core
N_CORES = 8
P = 128
DC = D // P    # 8 contraction chunks over d
ET = D // P    # 8 e-tiles
JT = S // P    # 16 key slots (global)
JTH = HALF // P  # 8 own tiles
I5 = HALF // 512  # 2

# query-tile ownership: interleaved (balances causal work, core-invariant
# union skip pattern). Core h owns global tiles 2g+h, g=0..7.
TILES = [[2 * g for g in range(JTH)], [2 * g + 1 for g in range(JTH)]]
# SBUF key slot s holds global key tile KEYTILE[s] (rank0 evens, rank1 odds)
KEYTILE = TILES[0] + TILES[1]

BF16 = ml_dtypes.bfloat16

_CACHE = {}


def _fkind(ib, s):
    """Classification of score block (ib, slot s), core-invariant.

    The reference mask KEEPS masked positions: m = (j > i) | pad_j.
    Blocks fully above the diagonal (all j > i) need no mask at all;
    blocks fully below need only the per-key padding column; the 8
    near-diagonal slots need a full per-core mask tile.
    """
    l = s % 8
    if l > 4 * ib + 3:
        return 'ones'
    return 'diag' if l >= 4 * ib else 'pad'


def _emit(ctx, tc, xq, wkt, wvt, wqt, maskd, padc, out,
          kt_own, kt_gath, v_own, v_gath):
    from concourse import mybir

    nc = tc.nc
    dtb = mybir.dt.bfloat16
    dtf = mybir.dt.float32
    dt8 = mybir.dt.float8e4

    # ---- SBUF pools -------------------------------------------------------
    xq_p = ctx.enter_context(tc.tile_pool(name="xq", bufs=1))
    # weights + scores blocks share one pool: all tiles are 16KB/partition
    w_p = ctx.enter_context(tc.tile_pool(name="wst", bufs=3))
    kt_p = ctx.enter_context(tc.tile_pool(name="kt", bufs=1))
    qt_p = ctx.enter_context(tc.tile_pool(name="qt", bufs=1))
    vp_p = ctx.enter_context(tc.tile_pool(name="vp", bufs=1))
    row_p = ctx.enter_context(tc.tile_pool(name="rows", bufs=1))
    sq_p = ctx.enter_context(tc.tile_pool(name="sq", bufs=3))
    stg_p = ctx.enter_context(tc.tile_pool(name="stg", bufs=10))
    mk_p = ctx.enter_context(tc.tile_pool(name="mk", bufs=4))
    ev_p = ctx.enter_context(tc.tile_pool(name="ev", bufs=3))
    ps_p = ctx.enter_context(tc.tile_pool(name="psmm", bufs=5, space="PSUM"))
    psr_p = ctx.enter_context(tc.tile_pool(name="psrow", bufs=2, space="PSUM"))
    psc_p = ctx.enter_context(tc.tile_pool(name="pscol", bufs=1, space="PSUM"))

    xq_s = xq_p.tile([P, I5 * DC * 512], dtb, tag="xq")  # [(j5 dc) tiled]
    wkt_s = w_p.tile([P, ET * DC * P], dtb, tag="w")     # [(et dc) tiled]
    wvt_s = w_p.tile([P, DC * D], dtb, tag="w")          # [d%128, dc*1024+e]
    wqt_s = w_p.tile([P, DC * D], dtb, tag="w")
    kt_s = kt_p.tile([P, ET * S], dtb, tag="kt")         # [e%128, et*2048+j]
    qt_s = qt_p.tile([P, ET * HALF], dtb, tag="qt")      # [e%128, et*1024+i]
    vp_s = vp_p.tile([P, JT * D], dtb, tag="vp")         # [j%128, s*1024+d]

    ones_b = row_p.tile([P, 1], dtb, tag="ones_b")
    ones_f = row_p.tile([1, 1], dtf, tag="ones_f")
    ksq_row = row_p.tile([1, HALF], dtf, tag="sqrow")
    qsq_row = row_p.tile([1, HALF], dtf, tag="sqrow")
    ksq_col = row_p.tile([P, JTH], dtf, tag="ksqc")    # col c <-> own j-chunk c
    krec_col = row_p.tile([P, JTH], dtf, tag="krecc")
    kinv_col = row_p.tile([P, JTH], dtf, tag="kinvc")
    qsq_col = row_p.tile([P, ET], dtf, tag="qsqc")
    qrec_col = row_p.tile([P, ET], dtf, tag="qrecc")
    qinv_col = row_p.tile([P, ET], dtf, tag="qinvc")
    pad_s = row_p.tile([P, JT], dtf, tag="pad")        # key padding per slot

    nc.vector.memset(ones_b[:], 1.0)
    nc.vector.memset(ones_f[:], 1.0)

    # ---- input DMAs, in consumption order across two HW queues -----------
    # B(j5=0,et=0) needs wkt tiles (et=0,*) + xq tiles (j5=0,*): the first
    # chunks are split across the sync and scalar queues so the first
    # projection chain can start ~3us in. Weights stream in behind on
    # scalar; the sync queue frees up early for kt_own/v_own stores.
    def xq_dma(q, j5, dclist):
        for dc in dclist:
            o = (j5 * DC + dc) * 512
            q.dma_start(xq_s[:, o:o + 512], xq[:, o:o + 512])
    nc.sync.dma_start(wkt_s[:, 0:DC * P], wkt[:, 0:DC * P])
    xq_dma(nc.scalar, 0, range(0, 4))
    xq_dma(nc.sync, 0, range(4, DC))
    # sync is now free for the kt_own/v_own stores feeding the AllGathers;
    # the remaining weight tiles stream on scalar just ahead of use.
    for et in range(1, ET):
        nc.scalar.dma_start(wkt_s[:, et * DC * P:(et + 1) * DC * P],
                            wkt[:, et * DC * P:(et + 1) * DC * P])
    xq_dma(nc.scalar, 1, range(0, DC))
    nc.scalar.dma_start(pad_s[:], padc[:, :])
    # wvt/wqt ride the otherwise-idle gpsimd queue (first gather load
    # only needs it after the first AllGather completes)
    for w_s, w_d in ((wvt_s, wvt), (wqt_s, wqt)):
        for dc in range(DC):
            nc.gpsimd.dma_start(w_s[:, dc * D:(dc + 1) * D],
                                w_d[dc * P:(dc + 1) * P, :])

    groups = [[0, 1], [2, 3], [4, 5], [6, 7]]

    def xq_rhs(j5, dc):      # [128, 512] own queries j5*512.. for d-chunk dc
        o = (j5 * DC + dc) * 512
        return xq_s[:, o:o + 512]

    def xq_lhsT(jt, dc):     # [128, 128] own queries jt*128.. for d-chunk dc
        o = ((jt // 4) * DC + dc) * 512 + (jt % 4) * P
        return xq_s[:, o:o + P]

    # ---- phase B: KT for own keys + k sumsq ------------------------------
    for j5 in range(I5):
        ksq_ps = psr_p.tile([1, 512], dtf, tag="psrow")
        for et in range(ET):
            ps = ps_p.tile([P, 512], dtf, tag="psmm")
            for dc in range(DC):
                nc.tensor.matmul(
                    ps[:],
                    lhsT=wkt_s[:, (et * DC + dc) * P:(et * DC + dc + 1) * P],
                    rhs=xq_rhs(j5, dc),
                    start=(dc == 0), stop=(dc == DC - 1),
                )
            stg = stg_p.tile([P, 512], dtb, tag="stg")
            nc.vector.tensor_copy(stg[:], ps[:])
            nc.sync.dma_start(kt_own[j5][et * P:(et + 1) * P, :], stg[:])
            sq = sq_p.tile([P, 512], dtb, tag="sq")
            nc.scalar.square(sq[:], ps[:])
            nc.tensor.matmul(ksq_ps[:], lhsT=ones_b[:], rhs=sq[:],
                             start=(et == 0), stop=(et == ET - 1))
        # gather this half of KT as soon as its stores land
        nc.gpsimd.collective_compute(
            "AllGather", mybir.AluOpType.bypass, replica_groups=groups,
            ins=[kt_own[j5][:]], outs=[kt_gath[j5][:]])
        nc.vector.tensor_copy(ksq_row[0:1, j5 * 512: j5 * 512 + 512], ksq_ps[:])
        for cc in range(4):
            c = j5 * 4 + cc
            pc = psc_p.tile([P, 1], dtf, tag="pscol")
            nc.tensor.matmul(pc[:], lhsT=ksq_row[0:1, c * P:(c + 1) * P],
                             rhs=ones_f[:], start=True, stop=True)
            nc.vector.tensor_copy(ksq_col[:, c:c + 1], pc[:])

    # kinv for own keys, 128-way parallel in column layout
    nc.vector.reciprocal(krec_col[:], ksq_col[:])
    nc.scalar.sqrt(kinv_col[:], krec_col[:])

    # load gathered KT halves to SBUF (slot-order j: rank r half, j5 piece)
    kt3 = kt_s[:].rearrange("p (et j) -> p et j", et=ET, j=S)
    for j5 in range(I5):
        for r in range(2):
            src3 = kt_gath[j5][r].rearrange("(et p) j -> p et j", p=P)
            for eg in range(0, ET, 4):
                nc.gpsimd.dma_start(
                    kt3[:, eg:eg + 4,
                        r * HALF + j5 * 512: r * HALF + j5 * 512 + 512],
                    src3[:, eg:eg + 4, :])

    # ---- phase D: V' = V * kinv[j] for own keys --------------------------
    # e5 pair shares the stationary xq tile per dc step (weight reuse)
    for jt in range(JTH):
        ps_a = ps_p.tile([P, 512], dtf, tag="psmm")
        ps_b = ps_p.tile([P, 512], dtf, tag="psmm")
        pspair = [ps_a, ps_b]
        for dc in range(DC):
            for e5 in range(2):
                nc.tensor.matmul(
                    pspair[e5][:],
                    lhsT=xq_lhsT(jt, dc),
                    rhs=wvt_s[:, dc * D + e5 * 512: dc * D + e5 * 512 + 512],
                    start=(dc == 0), stop=(dc == DC - 1),
                )
        for e5 in range(2):
            stg = stg_p.tile([P, 512], dtb, tag="stg")
            nc.vector.tensor_scalar_mul(stg[:], pspair[e5][:], kinv_col[:, jt:jt + 1])
            nc.sync.dma_start(
                v_own[jt // 4][(jt % 4) * P:(jt % 4 + 1) * P,
                               e5 * 512: e5 * 512 + 512], stg[:])
        if jt % 4 == 3:
            nc.gpsimd.collective_compute(
                "AllGather", mybir.AluOpType.bypass, replica_groups=groups,
                ins=[v_own[jt // 4][:]], outs=[v_gath[jt // 4][:]])

    for half in range(2):
        for r in range(2):
            o = (r * JTH + half * 4) * D
            dst = vp_s[:, o: o + 4 * D]
            dst = dst.rearrange("p (jtl e) -> p jtl e", jtl=4, e=D)
            src_ap = v_gath[half][r].rearrange("(jtl p) e -> p jtl e", p=P)
            nc.gpsimd.dma_start(dst[:, :, :], src_ap[:, :, :])

    # ---- phase E: QT + q sumsq -------------------------------------------
    for i5 in range(I5):
        qsq_ps = psr_p.tile([1, 512], dtf, tag="psrow")
        for et in range(ET):
            ps = ps_p.tile([P, 512], dtf, tag="psmm")
            for dc in range(DC):
                nc.tensor.matmul(
                    ps[:],
                    lhsT=wqt_s[:, dc * D + et * P: dc * D + (et + 1) * P],
                    rhs=xq_rhs(i5, dc),
                    start=(dc == 0), stop=(dc == DC - 1),
                )
            qtsl = qt_s[:, et * HALF + i5 * 512: et * HALF + i5 * 512 + 512]
            nc.vector.tensor_copy(qtsl, ps[:])
            sq = sq_p.tile([P, 512], dtb, tag="sq")
            nc.scalar.square(sq[:], ps[:])
            nc.tensor.matmul(qsq_ps[:], lhsT=ones_b[:], rhs=sq[:],
                             start=(et == 0), stop=(et == ET - 1))
        nc.vector.tensor_copy(qsq_row[0:1, i5 * 512: i5 * 512 + 512], qsq_ps[:])

    # ---- phase F: score blocks (union causal skip), then ------------------
    # ---- phase G: out blocks ---------------------------------------------
    st_blks = []
    for ib in range(I5):
        st_blk = w_p.tile([P, JT * 512], dtb, tag="w")  # [j%128, s*512+i]
        st_blks.append(st_blk)
        ndiag = 0
        for s in range(JT):
            kind = _fkind(ib, s)
            ps = ps_p.tile([P, 512], dtf, tag="psmm")
            for et in range(ET):
                nc.tensor.matmul(
                    ps[:],
                    lhsT=kt_s[:, et * S + s * P: et * S + (s + 1) * P],
                    rhs=qt_s[:, et * HALF + ib * 512: et * HALF + ib * 512 + 512],
                    start=(et == 0), stop=(et == ET - 1),
                )
            dst = st_blk[:, s * 512:(s + 1) * 512]
            if kind == 'diag':
                mk = mk_p.tile([P, 512], dt8, tag="mk")
                nc.scalar.dma_start(mk[:], maskd[(ib * 8 + ndiag) * P:
                                               (ib * 8 + ndiag + 1) * P, :])
                ndiag += 1
                nc.vector.tensor_mul(dst, ps[:], mk[:])
            elif kind == 'pad':
                nc.vector.tensor_scalar_mul(dst, ps[:], pad_s[:, s:s + 1])
            else:  # 'ones': fully above the diagonal, mask is all-ones
                nc.vector.tensor_copy(dst, ps[:])
    # q-norm chain (deferred so F's matmuls aren't blocked behind it)
    for c in range(ET):
        pc = psc_p.tile([P, 1], dtf, tag="pscol")
        nc.tensor.matmul(pc[:], lhsT=qsq_row[0:1, c * P:(c + 1) * P],
                         rhs=ones_f[:], start=True, stop=True)
        nc.vector.tensor_copy(qsq_col[:, c:c + 1], pc[:])
    nc.vector.reciprocal(qrec_col[:], qsq_col[:])
    nc.scalar.sqrt(qinv_col[:], qrec_col[:])

    for ib in range(I5):
        st_blk = st_blks[ib]
        for itl in range(4):
            g = ib * 4 + itl          # local i-tile
            for d5 in range(2):
                ps = ps_p.tile([P, 512], dtf, tag="psmm")
                for s in range(JT):
                    nc.tensor.matmul(
                        ps[:],
                        lhsT=st_blk[:, s * 512 + itl * P: s * 512 + (itl + 1) * P],
                        rhs=vp_s[:, s * D + d5 * 512: s * D + d5 * 512 + 512],
                        start=(s == 0), stop=(s == JT - 1),
                    )
                ot = ev_p.tile([P, 512], dtf, tag="ev")
                nc.vector.tensor_scalar_mul(ot[:], ps[:], qinv_col[:, g:g + 1])
                nc.sync.dma_start(out[g * P:(g + 1) * P, d5 * 512: d5 * 512 + 512],
                                  ot[:])


def _build():
    if "nc" in _CACHE:
        return _CACHE["nc"]
    import concourse.tile as tile
    from concourse import bacc, mybir

    dtb = mybir.dt.bfloat16
    dtf = mybir.dt.float32
    nc = bacc.Bacc("TRN2", target_bir_lowering=False, debug=False,
                   enable_asserts=True, num_devices=N_CORES)
    xq = nc.dram_tensor("xq", [P, I5 * DC * 512], dtb, kind="ExternalInput").ap()
    wkt = nc.dram_tensor("wkt", [P, ET * DC * P], dtb, kind="ExternalInput").ap()
    wvt = nc.dram_tensor("wvt", [D, D], dtb, kind="ExternalInput").ap()
    wqt = nc.dram_tensor("wqt", [D, D], dtb, kind="ExternalInput").ap()
    maskd = nc.dram_tensor("maskd", [I5 * 8 * P, 512],
                           mybir.dt.float8e4,
                           kind="ExternalInput").ap()
    padc = nc.dram_tensor("padc", [P, JT], dtf, kind="ExternalInput").ap()
    out = nc.dram_tensor("out", [HALF, D], dtf, kind="ExternalOutput").ap()
    kt_own = [nc.dram_tensor(f"kt_own{i}", [D, 512], dtb).ap() for i in range(2)]
    kt_gath = [nc.dram_tensor(f"kt_gath{i}", [2, D, 512], dtb).ap()
               for i in range(2)]
    v_own = [nc.dram_tensor(f"v_own{i}", [512, D], dtb).ap() for i in range(2)]
    v_gath = [nc.dram_tensor(f"v_gath{i}", [2, 512, D], dtb).ap()
              for i in range(2)]

    from contextlib import ExitStack
    with tile.TileContext(nc) as tc:
        with ExitStack() as ctx:
            _emit(ctx, tc, xq, wkt, wvt, wqt, maskd, padc, out,
                  kt_own, kt_gath, v_own, v_gath)
    nc.compile()
    _CACHE["nc"] = nc
    return nc


def make_in_maps(X, masks, Wq, Wk, Wv):
    """Host-side sharding/layout: one input map per core."""
    in_maps = []
    wkt_h = np.ascontiguousarray(Wk.T).astype(BF16)   # [D, D] = [d, e]
    wvt_h = np.ascontiguousarray(Wv.T).astype(BF16)
    wqt_h = np.ascontiguousarray(Wq.T).astype(BF16)
    # wkt pretiled: img[p, (et*DC+dc)*128 + c] = wkt[dc*128+p, et*128+c]
    wkt_img = np.ascontiguousarray(
        wkt_h.reshape(DC, P, ET, P).transpose(1, 2, 0, 3).reshape(P, ET * DC * P))
    keycols = np.concatenate(
        [np.arange(t * P, (t + 1) * P) for t in KEYTILE])   # slot-order keys
    for c in range(N_CORES):
        b, h = c // 2, c % 2
        XT = X[b].T.astype(BF16)                            # [D, S]
        qcols = np.concatenate(
            [np.arange(t * P, (t + 1) * P) for t in TILES[h]])
        xo = XT[:, qcols]                                   # [D, 1024]
        # xq pretiled: img[p, (j5*DC+dc)*512 + c] = xo[dc*128+p, j5*512+c]
        xq_img = np.ascontiguousarray(
            xo.reshape(DC, P, I5, 512).transpose(1, 2, 0, 3)
            .reshape(P, I5 * DC * 512))
        padbit = (masks[b] == 0)                            # True = padded key
        # near-diagonal mask blocks, in (ib, slot) emission order
        mrows = []
        for ib in range(I5):
            icols = qcols[ib * 512:(ib + 1) * 512]
            for s in range(JT):
                if _fkind(ib, s) != 'diag':
                    continue
                jrows = keycols[s * P:(s + 1) * P]
                m = ((jrows[:, None] > icols[None, :]) |
                     padbit[jrows][:, None]).astype(BF16)
                mrows.append(m)
        maskd_h = np.ascontiguousarray(
            np.concatenate(mrows, axis=0).astype(ml_dtypes.float8_e4m3fn))
        padc = np.ascontiguousarray(
            padbit[keycols].astype(np.float32).reshape(JT, P).T)  # [128, JT]
        in_maps.append({
            "xq": xq_img,
            "wkt": wkt_img,
            "wvt": wvt_h,
            "wqt": wqt_h,
            "maskd": maskd_h,
            "padc": padc,
        })
    return in_maps


def run(in_maps, **kw):
    from concourse.bass_utils import run_bass_kernel_spmd
    nc = _build()
    return run_bass_kernel_spmd(nc, in_maps, list(range(N_CORES)), **kw)


def kernel(X, masks, Wq, Wk, Wv):
    X = np.asarray(X, dtype=np.float32)
    masks = np.asarray(masks)
    res = run(make_in_maps(X, masks, np.asarray(Wq, np.float32),
                           np.asarray(Wk, np.float32), np.asarray(Wv, np.float32)))
    out = np.empty((B, S, D), np.float32)
    for c in range(N_CORES):
        b, h = c // 2, c % 2
        for g, t in enumerate(TILES[h]):
            out[b, t * P:(t + 1) * P, :] = res.results[c]["out"][g * P:(g + 1) * P]
    return out


# revision 16
# speedup vs baseline: 1.1216x; 1.0337x over previous
"""Bass/Trainium2 kernel for nn_Attention_75007308857927.

Reference computation (B=4, S=2048, D=1024):
    Q = X @ Wq.T ; K = X @ Wk.T ; V = X @ Wv.T         (per batch)
    Qn, Kn = row-normalized Q, K
    scores = (Qn @ Kn.T) * m      m[i,j] = 1 if (j > i) or masks[j]==0 else 0
    out = scores @ V

NOTE the mask orientation: m = 1 KEEPS a position (j > i, i.e. future,
or padded key j). Score blocks fully above the diagonal therefore need
no mask; blocks fully below need only a per-key padding column; only the
8 near-diagonal slots per 512-query block need a full mask tile.

Sharding: 8 cores = 4 batches x 2 query-tile sets, query tiles
interleaved (core h owns global tiles {2g+h}) so the near-diagonal
block set is core-invariant and one SPMD program serves both cores;
per-core mask data handles the rest. Keys stay with their owner,
AllGathered in slot order [rank0 tiles (even), rank1 tiles (odd)].

Device algebra per core (matmuls contract over the partition dim):
    KT[e,j']  = sum_d WkT[d,e] * XQ[d,j']        (own keys j')
    kinv[j']  = rsqrt(sum_e KT[e,j']^2)
    V'[j',e]  = (sum_d XQ[d,j'] WvT[d,e]) * kinv[j']
    KT, V'    = AllGather over the core pair     (slot order)
    QT[e,i]   = sum_d WqT[d,e] * XQ[d,i]
    qinv[i]   = rsqrt(sum_e QT[e,i]^2)
    ST[j,i]   = (sum_e KT[e,j] QT[e,i]) * maskT[j,i]   (skipped above diag;
                pad-only column mask off-diagonal, full mask near diagonal)
    out[i,d]  = (sum_{j in slots(g)} ST[j,i] V'[j,d]) * qinv[i]

Weights and XQ are host-pretiled into SBUF image order so the first
projection chain can start after ~1.25MB of DMA.

bf16 matmul operands, f32 PSUM accumulation. Precision vs f32 reference:
absmax error ~0.4% of output scale.
"""

import numpy as np
import ml_dtypes

B, S, D = 4, 2048, 1024
HALF = S // 2  # queries/keys per core
N_CORES = 8
P = 128
DC = D // P    # 8 contraction chunks over d
ET = D // P    # 8 e-tiles
JT = S // P    # 16 key slots (global)
JTH = HALF // P  # 8 own tiles
I5 = HALF // 512  # 2

# query-tile ownership: interleaved (balances causal work, core-invariant
# union skip pattern). Core h owns global tiles 2g+h, g=0..7.
TILES = [[2 * g for g in range(JTH)], [2 * g + 1 for g in range(JTH)]]
# SBUF key slot s holds global key tile KEYTILE[s] (rank0 evens, rank1 odds)
KEYTILE = TILES[0] + TILES[1]

BF16 = ml_dtypes.bfloat16

_CACHE = {}


def _fkind(ib, s):
    """Classification of score block (ib, slot s), core-invariant.

    The reference mask KEEPS masked positions: m = (j > i) | pad_j.
    Blocks fully above the diagonal (all j > i) need no mask at all;
    blocks fully below need only the per-key padding column; the 8
    near-diagonal slots need a full per-core mask tile.
    """
    l = s % 8
    if l > 4 * ib + 3:
        return 'ones'
    return 'diag' if l >= 4 * ib else 'pad'


def _emit(ctx, tc, xq, wkt, wvt, wqt, maskd, padc, out,
          kt_own, kt_gath, v_own, v_gath):
    from concourse import mybir

    nc = tc.nc
    dtb = mybir.dt.bfloat16
    dtf = mybir.dt.float32
    dt8 = mybir.dt.float8e4

    # ---- SBUF pools -------------------------------------------------------
    xq_p = ctx.enter_context(tc.tile_pool(name="xq", bufs=1))
    # weights + scores blocks share one pool: all tiles are 16KB/partition
    w_p = ctx.enter_context(tc.tile_pool(name="wst", bufs=3))
    kt_p = ctx.enter_context(tc.tile_pool(name="kt", bufs=1))
    qt_p = ctx.enter_context(tc.tile_pool(name="qt", bufs=1))
    vp_p = ctx.enter_context(tc.tile_pool(name="vp", bufs=1))
    row_p = ctx.enter_context(tc.tile_pool(name="rows", bufs=1))
    sq_p = ctx.enter_context(tc.tile_pool(name="sq", bufs=3))
    stg_p = ctx.enter_context(tc.tile_pool(name="stg", bufs=10))
    mk_p = ctx.enter_context(tc.tile_pool(name="mk", bufs=4))
    ev_p = ctx.enter_context(tc.tile_pool(name="ev", bufs=3))
    ps_p = ctx.enter_context(tc.tile_pool(name="psmm", bufs=5, space="PSUM"))
    psr_p = ctx.enter_context(tc.tile_pool(name="psrow", bufs=2, space="PSUM"))
    psc_p = ctx.enter_context(tc.tile_pool(name="pscol", bufs=1, space="PSUM"))

    xq_s = xq_p.tile([P, I5 * DC * 512], dtb, tag="xq")  # [(j5 dc) tiled]
    wkt_s = w_p.tile([P, ET * DC * P], dtb, tag="w")     # [(et dc) tiled]
    wvt_s = w_p.tile([P, DC * D], dtb, tag="w")          # [d%128, dc*1024+e]
    wqt_s = w_p.tile([P, DC * D], dtb, tag="w")
    kt_s = kt_p.tile([P, ET * S], dtb, tag="kt")         # [e%128, et*2048+j]
    qt_s = qt_p.tile([P, ET * HALF], dtb, tag="qt")      # [e%128, et*1024+i]
    vp_s = vp_p.tile([P, JT * D], dtb, tag="vp")         # [j%128, s*1024+d]

    ones_b = row_p.tile([P, 1], dtb, tag="ones_b")
    ones_f = row_p.tile([1, 1], dtf, tag="ones_f")
    ksq_row = row_p.tile([1, HALF], dtf, tag="sqrow")
    qsq_row = row_p.tile([1, HALF], dtf, tag="sqrow")
    ksq_col = row_p.tile([P, JTH], dtf, tag="ksqc")    # col c <-> own j-chunk c
    krec_col = row_p.tile([P, JTH], dtf, tag="krecc")
    kinv_col = row_p.tile([P, JTH], dtf, tag="kinvc")
    qsq_col = row_p.tile([P, ET], dtf, tag="qsqc")
    qrec_col = row_p.tile([P, ET], dtf, tag="qrecc")
    qinv_col = row_p.tile([P, ET], dtf, tag="qinvc")
    pad_s = row_p.tile([P, JT], dtf, tag="pad")        # key padding per slot

    nc.vector.memset(ones_b[:], 1.0)
    nc.vector.memset(ones_f[:], 1.0)

    # ---- input DMAs, in consumption order across two HW queues -----------
    # B(j5=0,et=0) needs wkt tiles (et=0,*) + xq tiles (j5=0,*): the first
    # chunks are split across the sync and scalar queues so the first
    # projection chain can start ~3us in. Weights stream in behind on
    # scalar; the sync queue frees up early for kt_own/v_own stores.
    def xq_dma(q, j5, dclist):
        for dc in dclist:
            o = (j5 * DC + dc) * 512
            q.dma_start(xq_s[:, o:o + 512], xq[:, o:o + 512])
    nc.sync.dma_start(wkt_s[:, 0:DC * P], wkt[:, 0:DC * P])
    xq_dma(nc.scalar, 0, range(0, 4))
    xq_dma(nc.sync, 0, range(4, DC))
    # sync is now free for the kt_own/v_own stores feeding the AllGathers;
    # the remaining weight tiles stream on scalar just ahead of use.
    for et in range(1, ET):
        nc.scalar.dma_start(wkt_s[:, et * DC * P:(et + 1) * DC * P],
                            wkt[:, et * DC * P:(et + 1) * DC * P])
    nc.scalar.dma_start(pad_s[:], padc[:, :])
    # xq second half + wvt ride the otherwise-idle gpsimd queue. wqt and
    # the gather loads are emitted after phase B so the AllGather triggers
    # are not queued behind their descriptor generation.
    xq_dma(nc.gpsimd, 1, range(0, DC))
    for dc in range(DC):
        nc.gpsimd.dma_start(wvt_s[:, dc * D:(dc + 1) * D],
                            wvt[dc * P:(dc + 1) * P, :])

    groups = [[0, 1], [2, 3], [4, 5], [6, 7]]

    def xq_rhs(j5, dc):      # [128, 512] own queries j5*512.. for d-chunk dc
        o = (j5 * DC + dc) * 512
        return xq_s[:, o:o + 512]

    def xq_lhsT(jt, dc):     # [128, 128] own queries jt*128.. for d-chunk dc
        o = ((jt // 4) * DC + dc) * 512 + (jt % 4) * P
        return xq_s[:, o:o + P]

    # ---- phase B: KT for own keys + k sumsq ------------------------------
    for j5 in range(I5):
        ksq_ps = psr_p.tile([1, 512], dtf, tag="psrow")
        for et in range(ET):
            ps = ps_p.tile([P, 512], dtf, tag="psmm")
            for dc in range(DC):
                nc.tensor.matmul(
                    ps[:],
                    lhsT=wkt_s[:, (et * DC + dc) * P:(et * DC + dc + 1) * P],
                    rhs=xq_rhs(j5, dc),
                    start=(dc == 0), stop=(dc == DC - 1),
                )
            stg = stg_p.tile([P, 512], dtb, tag="stg")
            nc.vector.tensor_copy(stg[:], ps[:])
            nc.sync.dma_start(kt_own[j5][et * P:(et + 1) * P, :], stg[:])
            sq = sq_p.tile([P, 512], dtb, tag="sq")
            nc.scalar.square(sq[:], ps[:])
            nc.tensor.matmul(ksq_ps[:], lhsT=ones_b[:], rhs=sq[:],
                             start=(et == 0), stop=(et == ET - 1))
        # gather this half of KT as soon as its stores land
        nc.gpsimd.collective_compute(
            "AllGather", mybir.AluOpType.bypass, replica_groups=groups,
            ins=[kt_own[j5][:]], outs=[kt_gath[j5][:]])
        nc.vector.tensor_copy(ksq_row[0:1, j5 * 512: j5 * 512 + 512], ksq_ps[:])
        for cc in range(4):
            c = j5 * 4 + cc
            pc = psc_p.tile([P, 1], dtf, tag="pscol")
            nc.tensor.matmul(pc[:], lhsT=ksq_row[0:1, c * P:(c + 1) * P],
                             rhs=ones_f[:], start=True, stop=True)
            nc.vector.tensor_copy(ksq_col[:, c:c + 1], pc[:])

    # kinv for own keys, 128-way parallel in column layout
    nc.vector.reciprocal(krec_col[:], ksq_col[:])
    nc.scalar.sqrt(kinv_col[:], krec_col[:])

    for dc in range(DC):
        nc.gpsimd.dma_start(wqt_s[:, dc * D:(dc + 1) * D],
                            wqt[dc * P:(dc + 1) * P, :])
    # load gathered KT halves to SBUF (slot-order j: rank r half, j5 piece)
    kt3 = kt_s[:].rearrange("p (et j) -> p et j", et=ET, j=S)
    for j5 in range(I5):
        for r in range(2):
            src3 = kt_gath[j5][r].rearrange("(et p) j -> p et j", p=P)
            for eg in range(0, ET, 4):
                nc.gpsimd.dma_start(
                    kt3[:, eg:eg + 4,
                        r * HALF + j5 * 512: r * HALF + j5 * 512 + 512],
                    src3[:, eg:eg + 4, :])

    # ---- phase D: V' = V * kinv[j] for own keys --------------------------
    # e5 pair shares the stationary xq tile per dc step (weight reuse)
    for jt in range(JTH):
        ps_a = ps_p.tile([P, 512], dtf, tag="psmm")
        ps_b = ps_p.tile([P, 512], dtf, tag="psmm")
        pspair = [ps_a, ps_b]
        for dc in range(DC):
            for e5 in range(2):
                nc.tensor.matmul(
                    pspair[e5][:],
                    lhsT=xq_lhsT(jt, dc),
                    rhs=wvt_s[:, dc * D + e5 * 512: dc * D + e5 * 512 + 512],
                    start=(dc == 0), stop=(dc == DC - 1),
                )
        for e5 in range(2):
            stg = stg_p.tile([P, 512], dtb, tag="stg")
            nc.vector.tensor_scalar_mul(stg[:], pspair[e5][:], kinv_col[:, jt:jt + 1])
            nc.sync.dma_start(
                v_own[jt // 4][(jt % 4) * P:(jt % 4 + 1) * P,
                               e5 * 512: e5 * 512 + 512], stg[:])
        if jt % 4 == 3:
            nc.gpsimd.collective_compute(
                "AllGather", mybir.AluOpType.bypass, replica_groups=groups,
                ins=[v_own[jt // 4][:]], outs=[v_gath[jt // 4][:]])

    for half in range(2):
        for r in range(2):
            o = (r * JTH + half * 4) * D
            dst = vp_s[:, o: o + 4 * D]
            dst = dst.rearrange("p (jtl e) -> p jtl e", jtl=4, e=D)
            src_ap = v_gath[half][r].rearrange("(jtl p) e -> p jtl e", p=P)
            nc.gpsimd.dma_start(dst[:, :, :], src_ap[:, :, :])

    # ---- phase E: QT + q sumsq -------------------------------------------
    for i5 in range(I5):
        qsq_ps = psr_p.tile([1, 512], dtf, tag="psrow")
        for et in range(ET):
            ps = ps_p.tile([P, 512], dtf, tag="psmm")
            for dc in range(DC):
                nc.tensor.matmul(
                    ps[:],
                    lhsT=wqt_s[:, dc * D + et * P: dc * D + (et + 1) * P],
                    rhs=xq_rhs(i5, dc),
                    start=(dc == 0), stop=(dc == DC - 1),
                )
            qtsl = qt_s[:, et * HALF + i5 * 512: et * HALF + i5 * 512 + 512]
            nc.vector.tensor_copy(qtsl, ps[:])
            sq = sq_p.tile([P, 512], dtb, tag="sq")
            nc.scalar.square(sq[:], ps[:])
            nc.tensor.matmul(qsq_ps[:], lhsT=ones_b[:], rhs=sq[:],
                             start=(et == 0), stop=(et == ET - 1))
        nc.vector.tensor_copy(qsq_row[0:1, i5 * 512: i5 * 512 + 512], qsq_ps[:])

    # ---- phase F: score blocks (union causal skip), then ------------------
    # ---- phase G: out blocks ---------------------------------------------
    st_blks = []
    for ib in range(I5):
        st_blk = w_p.tile([P, JT * 512], dtb, tag="w")  # [j%128, s*512+i]
        st_blks.append(st_blk)
        ndiag = 0
        for s in range(JT):
            kind = _fkind(ib, s)
            ps = ps_p.tile([P, 512], dtf, tag="psmm")
            for et in range(ET):
                nc.tensor.matmul(
                    ps[:],
                    lhsT=kt_s[:, et * S + s * P: et * S + (s + 1) * P],
                    rhs=qt_s[:, et * HALF + ib * 512: et * HALF + ib * 512 + 512],
                    start=(et == 0), stop=(et == ET - 1),
                )
            dst = st_blk[:, s * 512:(s + 1) * 512]
            if kind == 'diag':
                mk = mk_p.tile([P, 512], dt8, tag="mk")
                nc.scalar.dma_start(mk[:], maskd[(ib * 8 + ndiag) * P:
                                               (ib * 8 + ndiag + 1) * P, :])
                ndiag += 1
                nc.vector.tensor_mul(dst, ps[:], mk[:])
            elif kind == 'pad':
                nc.vector.tensor_scalar_mul(dst, ps[:], pad_s[:, s:s + 1])
            else:  # 'ones': fully above the diagonal, mask is all-ones
                nc.vector.tensor_copy(dst, ps[:])
    # q-norm chain (deferred so F's matmuls aren't blocked behind it)
    for c in range(ET):
        pc = psc_p.tile([P, 1], dtf, tag="pscol")
        nc.tensor.matmul(pc[:], lhsT=qsq_row[0:1, c * P:(c + 1) * P],
                         rhs=ones_f[:], start=True, stop=True)
        nc.vector.tensor_copy(qsq_col[:, c:c + 1], pc[:])
    nc.vector.reciprocal(qrec_col[:], qsq_col[:])
    nc.scalar.sqrt(qinv_col[:], qrec_col[:])

    for ib in range(I5):
        st_blk = st_blks[ib]
        for itl in range(4):
            g = ib * 4 + itl          # local i-tile
            for d5 in range(2):
                ps = ps_p.tile([P, 512], dtf, tag="psmm")
                for s in range(JT):
                    nc.tensor.matmul(
                        ps[:],
                        lhsT=st_blk[:, s * 512 + itl * P: s * 512 + (itl + 1) * P],
                        rhs=vp_s[:, s * D + d5 * 512: s * D + d5 * 512 + 512],
                        start=(s == 0), stop=(s == JT - 1),
                    )
                ot = ev_p.tile([P, 512], dtf, tag="ev")
                nc.vector.tensor_scalar_mul(ot[:], ps[:], qinv_col[:, g:g + 1])
                nc.sync.dma_start(out[g * P:(g + 1) * P, d5 * 512: d5 * 512 + 512],
                                  ot[:])


def _build():
    if "nc" in _CACHE:
        return _CACHE["nc"]
    import concourse.tile as tile
    from concourse import bacc, mybir

    dtb = mybir.dt.bfloat16
    dtf = mybir.dt.float32
    nc = bacc.Bacc("TRN2", target_bir_lowering=False, debug=False,
                   enable_asserts=True, num_devices=N_CORES)
    xq = nc.dram_tensor("xq", [P, I5 * DC * 512], dtb, kind="ExternalInput").ap()
    wkt = nc.dram_tensor("wkt", [P, ET * DC * P], dtb, kind="ExternalInput").ap()
    wvt = nc.dram_tensor("wvt", [D, D], dtb, kind="ExternalInput").ap()
    wqt = nc.dram_tensor("wqt", [D, D], dtb, kind="ExternalInput").ap()
    maskd = nc.dram_tensor("maskd", [I5 * 8 * P, 512],
                           mybir.dt.float8e4,
                           kind="ExternalInput").ap()
    padc = nc.dram_tensor("padc", [P, JT], dtf, kind="ExternalInput").ap()
    out = nc.dram_tensor("out", [HALF, D], dtf, kind="ExternalOutput").ap()
    kt_own = [nc.dram_tensor(f"kt_own{i}", [D, 512], dtb).ap() for i in range(2)]
    kt_gath = [nc.dram_tensor(f"kt_gath{i}", [2, D, 512], dtb).ap()
               for i in range(2)]
    v_own = [nc.dram_tensor(f"v_own{i}", [512, D], dtb).ap() for i in range(2)]
    v_gath = [nc.dram_tensor(f"v_gath{i}", [2, 512, D], dtb).ap()
              for i in range(2)]

    from contextlib import ExitStack
    with tile.TileContext(nc) as tc:
        with ExitStack() as ctx:
            _emit(ctx, tc, xq, wkt, wvt, wqt, maskd, padc, out,
                  kt_own, kt_gath, v_own, v_gath)
    nc.compile()
    _CACHE["nc"] = nc
    return nc


def make_in_maps(X, masks, Wq, Wk, Wv):
    """Host-side sharding/layout: one input map per core."""
    in_maps = []
    wkt_h = np.ascontiguousarray(Wk.T).astype(BF16)   # [D, D] = [d, e]
    wvt_h = np.ascontiguousarray(Wv.T).astype(BF16)
    wqt_h = np.ascontiguousarray(Wq.T).astype(BF16)
    # wkt pretiled: img[p, (et*DC+dc)*128 + c] = wkt[dc*128+p, et*128+c]
    wkt_img = np.ascontiguousarray(
        wkt_h.reshape(DC, P, ET, P).transpose(1, 2, 0, 3).reshape(P, ET * DC * P))
    keycols = np.concatenate(
        [np.arange(t * P, (t + 1) * P) for t in KEYTILE])   # slot-order keys
    for c in range(N_CORES):
        b, h = c // 2, c % 2
        XT = X[b].T.astype(BF16)                            # [D, S]
        qcols = np.concatenate(
            [np.arange(t * P, (t + 1) * P) for t in TILES[h]])
        xo = XT[:, qcols]                                   # [D, 1024]
        # xq pretiled: img[p, (j5*DC+dc)*512 + c] = xo[dc*128+p, j5*512+c]
        xq_img = np.ascontiguousarray(
            xo.reshape(DC, P, I5, 512).transpose(1, 2, 0, 3)
            .reshape(P, I5 * DC * 512))
        padbit = (masks[b] == 0)                            # True = padded key
        # near-diagonal mask blocks, in (ib, slot) emission order
        mrows = []
        for ib in range(I5):
            icols = qcols[ib * 512:(ib + 1) * 512]
            for s in range(JT):
                if _fkind(ib, s) != 'diag':
                    continue
                jrows = keycols[s * P:(s + 1) * P]
                m = ((jrows[:, None] > icols[None, :]) |
                     padbit[jrows][:, None]).astype(BF16)
                mrows.append(m)
        maskd_h = np.ascontiguousarray(
            np.concatenate(mrows, axis=0).astype(ml_dtypes.float8_e4m3fn))
        padc = np.ascontiguousarray(
            padbit[keycols].astype(np.float32).reshape(JT, P).T)  # [128, JT]
        in_maps.append({
            "xq": xq_img,
            "wkt": wkt_img,
            "wvt": wvt_h,
            "wqt": wqt_h,
            "maskd": maskd_h,
            "padc": padc,
        })
    return in_maps


def run(in_maps, **kw):
    from concourse.bass_utils import run_bass_kernel_spmd
    nc = _build()
    return run_bass_kernel_spmd(nc, in_maps, list(range(N_CORES)), **kw)


def kernel(X, masks, Wq, Wk, Wv):
    X = np.asarray(X, dtype=np.float32)
    masks = np.asarray(masks)
    res = run(make_in_maps(X, masks, np.asarray(Wq, np.float32),
                           np.asarray(Wk, np.float32), np.asarray(Wv, np.float32)))
    out = np.empty((B, S, D), np.float32)
    for c in range(N_CORES):
        b, h = c // 2, c % 2
        for g, t in enumerate(TILES[h]):
            out[b, t * P:(t + 1) * P, :] = res.results[c]["out"][g * P:(g + 1) * P]
    return out


# revision 19
# speedup vs baseline: 1.1325x; 1.0097x over previous
"""Bass/Trainium2 kernel for nn_Attention_75007308857927.

Reference computation (B=4, S=2048, D=1024):
    Q = X @ Wq.T ; K = X @ Wk.T ; V = X @ Wv.T         (per batch)
    Qn, Kn = row-normalized Q, K
    scores = (Qn @ Kn.T) * m      m[i,j] = 1 if (j > i) or masks[j]==0 else 0
    out = scores @ V

NOTE the mask orientation: m = 1 KEEPS a position (j > i, i.e. future,
or padded key j). Score blocks fully above the diagonal therefore need
no mask; blocks fully below need only a per-key padding column; only the
8 near-diagonal slots per 512-query block need a full mask tile.

Sharding: 8 cores = 4 batches x 2 query-tile sets, query tiles
interleaved (core h owns global tiles {2g+h}) so the near-diagonal
block set is core-invariant and one SPMD program serves both cores;
per-core mask data handles the rest. Keys stay with their owner,
AllGathered in slot order [rank0 tiles (even), rank1 tiles (odd)].

Device algebra per core (matmuls contract over the partition dim):
    KT[e,j']  = sum_d WkT[d,e] * XQ[d,j']        (own keys j')
    kinv[j']  = rsqrt(sum_e KT[e,j']^2)
    V'[j',e]  = (sum_d XQ[d,j'] WvT[d,e]) * kinv[j']
    KT, V'    = AllGather over the core pair     (slot order)
    QT[e,i]   = sum_d WqT[d,e] * XQ[d,i]
    qinv[i]   = rsqrt(sum_e QT[e,i]^2)
    ST[j,i]   = (sum_e KT[e,j] QT[e,i]) * maskT[j,i]   (skipped above diag;
                pad-only column mask off-diagonal, full mask near diagonal)
    out[i,d]  = (sum_{j in slots(g)} ST[j,i] V'[j,d]) * qinv[i]

Weights and XQ are host-pretiled into SBUF image order so the first
projection chain can start after ~1.25MB of DMA.

bf16 matmul operands, f32 PSUM accumulation. Precision vs f32 reference:
absmax error ~0.4% of output scale.
"""

import numpy as np
import ml_dtypes

B, S, D = 4, 2048, 1024
HALF = S // 2  # queries/keys per core
N_CORES = 8
P = 128
DC = D // P    # 8 contraction chunks over d
ET = D // P    # 8 e-tiles
JT = S // P    # 16 key slots (global)
JTH = HALF // P  # 8 own tiles
I5 = HALF // 512  # 2

# query-tile ownership: interleaved (balances causal work, core-invariant
# union skip pattern). Core h owns global tiles 2g+h, g=0..7.
TILES = [[2 * g for g in range(JTH)], [2 * g + 1 for g in range(JTH)]]
# SBUF key slot s holds global key tile KEYTILE[s] (rank0 evens, rank1 odds)
KEYTILE = TILES[0] + TILES[1]

BF16 = ml_dtypes.bfloat16

_CACHE = {}


def _fkind(ib, s):
    """Classification of score block (ib, slot s), core-invariant.

    The reference mask KEEPS masked positions: m = (j > i) | pad_j.
    Blocks fully above the diagonal (all j > i) need no mask at all;
    blocks fully below need only the per-key padding column; the 8
    near-diagonal slots need a full per-core mask tile.
    """
    l = s % 8
    if l > 4 * ib + 3:
        return 'ones'
    return 'diag' if l >= 4 * ib else 'pad'


def _emit(ctx, tc, xq, wkt, wvt, wqt, maskd, padc, out,
          kt_own, kt_gath, v_own, v_gath):
    from concourse import mybir

    nc = tc.nc
    dtb = mybir.dt.bfloat16
    dtf = mybir.dt.float32
    dt8 = mybir.dt.float8e4

    # ---- SBUF pools -------------------------------------------------------
    xq_p = ctx.enter_context(tc.tile_pool(name="xq", bufs=1))
    # weights + scores blocks share one pool: all tiles are 16KB/partition
    w_p = ctx.enter_context(tc.tile_pool(name="wst", bufs=3))
    kt_p = ctx.enter_context(tc.tile_pool(name="kt", bufs=1))
    qt_p = ctx.enter_context(tc.tile_pool(name="qt", bufs=1))
    vp_p = ctx.enter_context(tc.tile_pool(name="vp", bufs=1))
    row_p = ctx.enter_context(tc.tile_pool(name="rows", bufs=1))
    sq_p = ctx.enter_context(tc.tile_pool(name="sq", bufs=6))
    stg_p = ctx.enter_context(tc.tile_pool(name="stg", bufs=10))
    mk_p = ctx.enter_context(tc.tile_pool(name="mk", bufs=4))
    ev_p = ctx.enter_context(tc.tile_pool(name="ev", bufs=3))
    ps_p = ctx.enter_context(tc.tile_pool(name="psmm", bufs=5, space="PSUM"))
    psr_p = ctx.enter_context(tc.tile_pool(name="psrow", bufs=2, space="PSUM"))
    psc_p = ctx.enter_context(tc.tile_pool(name="pscol", bufs=1, space="PSUM"))

    xq_s = xq_p.tile([P, I5 * DC * 512], dtb, tag="xq")  # [(j5 dc) tiled]
    wkt_s = w_p.tile([P, ET * DC * P], dtb, tag="w")     # [(et dc) tiled]
    wvt_s = w_p.tile([P, DC * D], dtb, tag="w")          # [d%128, dc*1024+e]
    wqt_s = w_p.tile([P, DC * D], dtb, tag="w")
    kt_s = kt_p.tile([P, ET * S], dtb, tag="kt")         # [e%128, et*2048+j]
    qt_s = qt_p.tile([P, ET * HALF], dtb, tag="qt")      # [e%128, et*1024+i]
    vp_s = vp_p.tile([P, JT * D], dtb, tag="vp")         # [j%128, s*1024+d]

    ones_b = row_p.tile([P, 1], dtb, tag="ones_b")
    ones_f = row_p.tile([1, 1], dtf, tag="ones_f")
    ksq_row = row_p.tile([1, HALF], dtf, tag="sqrow")
    qsq_row = row_p.tile([1, HALF], dtf, tag="sqrow")
    ksq_col = row_p.tile([P, JTH], dtf, tag="ksqc")    # col c <-> own j-chunk c
    krec_col = row_p.tile([P, JTH], dtf, tag="krecc")
    kinv_col = row_p.tile([P, JTH], dtf, tag="kinvc")
    qsq_col = row_p.tile([P, ET], dtf, tag="qsqc")
    qrec_col = row_p.tile([P, ET], dtf, tag="qrecc")
    qinv_col = row_p.tile([P, ET], dtf, tag="qinvc")
    pad_s = row_p.tile([P, JT], dtf, tag="pad")        # key padding per slot

    nc.vector.memset(ones_b[:], 1.0)
    nc.vector.memset(ones_f[:], 1.0)

    # ---- input DMAs, in consumption order across two HW queues -----------
    # B(j5=0,et=0) needs wkt tiles (et=0,*) + xq tiles (j5=0,*): the first
    # chunks are split across the sync and scalar queues so the first
    # projection chain can start ~3us in. Weights stream in behind on
    # scalar; the sync queue frees up early for kt_own/v_own stores.
    def xq_dma(q, j5, dclist):
        for dc in dclist:
            o = (j5 * DC + dc) * 512
            q.dma_start(xq_s[:, o:o + 512], xq[:, o:o + 512])
    nc.sync.dma_start(wkt_s[:, 0:DC * P], wkt[:, 0:DC * P])
    xq_dma(nc.scalar, 0, range(0, 4))
    xq_dma(nc.sync, 0, range(4, DC))
    # sync is now free for the kt_own/v_own stores feeding the AllGathers;
    # the remaining weight tiles stream on scalar just ahead of use.
    for et in range(1, ET):
        nc.scalar.dma_start(wkt_s[:, et * DC * P:(et + 1) * DC * P],
                            wkt[:, et * DC * P:(et + 1) * DC * P])
    nc.scalar.dma_start(pad_s[:], padc[:, :])
    # xq second half + wvt ride the otherwise-idle gpsimd queue. wqt and
    # the gather loads are emitted after phase B so the AllGather triggers
    # are not queued behind their descriptor generation.
    xq_dma(nc.gpsimd, 1, range(0, DC))
    for dc in range(DC):
        nc.gpsimd.dma_start(wvt_s[:, dc * D:(dc + 1) * D],
                            wvt[dc * P:(dc + 1) * P, :])

    groups = [[0, 1], [2, 3], [4, 5], [6, 7]]

    def xq_rhs(j5, dc):      # [128, 512] own queries j5*512.. for d-chunk dc
        o = (j5 * DC + dc) * 512
        return xq_s[:, o:o + 512]

    def xq_lhsT(jt, dc):     # [128, 128] own queries jt*128.. for d-chunk dc
        o = ((jt // 4) * DC + dc) * 512 + (jt % 4) * P
        return xq_s[:, o:o + P]

    # ---- phase B: KT for own keys + k sumsq ------------------------------
    # the sumsq ones-matmul for e-tile k is emitted after chain k+1 so the
    # ACT square has a full chain of slack before the PE needs it
    for j5 in range(I5):
        ksq_ps = psr_p.tile([1, 512], dtf, tag="psrow")
        sqs = []
        for et in range(ET):
            ps = ps_p.tile([P, 512], dtf, tag="psmm")
            for dc in range(DC):
                nc.tensor.matmul(
                    ps[:],
                    lhsT=wkt_s[:, (et * DC + dc) * P:(et * DC + dc + 1) * P],
                    rhs=xq_rhs(j5, dc),
                    start=(dc == 0), stop=(dc == DC - 1),
                )
            stg = stg_p.tile([P, 512], dtb, tag="stg")
            nc.vector.tensor_copy(stg[:], ps[:])
            nc.sync.dma_start(kt_own[j5][et * P:(et + 1) * P, :], stg[:])
            sq = sq_p.tile([P, 512], dtb, tag="sq")
            nc.scalar.square(sq[:], ps[:])
            sqs.append(sq)
            if et > 0:
                nc.tensor.matmul(ksq_ps[:], lhsT=ones_b[:], rhs=sqs[et - 1][:],
                                 start=(et == 1), stop=False)
        nc.tensor.matmul(ksq_ps[:], lhsT=ones_b[:], rhs=sqs[ET - 1][:],
                         start=False, stop=True)
        # gather this half of KT as soon as its stores land
        nc.gpsimd.collective_compute(
            "AllGather", mybir.AluOpType.bypass, replica_groups=groups,
            ins=[kt_own[j5][:]], outs=[kt_gath[j5][:]])
        nc.vector.tensor_copy(ksq_row[0:1, j5 * 512: j5 * 512 + 512], ksq_ps[:])
        for cc in range(4):
            c = j5 * 4 + cc
            pc = psc_p.tile([P, 1], dtf, tag="pscol")
            nc.tensor.matmul(pc[:], lhsT=ksq_row[0:1, c * P:(c + 1) * P],
                             rhs=ones_f[:], start=True, stop=True)
            nc.vector.tensor_copy(ksq_col[:, c:c + 1], pc[:])

    # kinv for own keys, 128-way parallel in column layout
    nc.vector.reciprocal(krec_col[:], ksq_col[:])
    nc.scalar.sqrt(kinv_col[:], krec_col[:])

    for dc in range(DC):
        nc.gpsimd.dma_start(wqt_s[:, dc * D:(dc + 1) * D],
                            wqt[dc * P:(dc + 1) * P, :])
    # load gathered KT halves to SBUF (slot-order j: rank r half, j5 piece)
    kt3 = kt_s[:].rearrange("p (et j) -> p et j", et=ET, j=S)
    for j5 in range(I5):
        for r in range(2):
            src3 = kt_gath[j5][r].rearrange("(et p) j -> p et j", p=P)
            for eg in range(0, ET, 4):
                nc.gpsimd.dma_start(
                    kt3[:, eg:eg + 4,
                        r * HALF + j5 * 512: r * HALF + j5 * 512 + 512],
                    src3[:, eg:eg + 4, :])

    # ---- phase D: V' = V * kinv[j] for own keys --------------------------
    # e5 pair shares the stationary xq tile per dc step (weight reuse)
    for jt in range(JTH):
        ps_a = ps_p.tile([P, 512], dtf, tag="psmm")
        ps_b = ps_p.tile([P, 512], dtf, tag="psmm")
        pspair = [ps_a, ps_b]
        for dc in range(DC):
            for e5 in range(2):
                nc.tensor.matmul(
                    pspair[e5][:],
                    lhsT=xq_lhsT(jt, dc),
                    rhs=wvt_s[:, dc * D + e5 * 512: dc * D + e5 * 512 + 512],
                    start=(dc == 0), stop=(dc == DC - 1),
                )
        for e5 in range(2):
            stg = stg_p.tile([P, 512], dtb, tag="stg")
            nc.vector.tensor_scalar_mul(stg[:], pspair[e5][:], kinv_col[:, jt:jt + 1])
            nc.sync.dma_start(
                v_own[jt // 4][(jt % 4) * P:(jt % 4 + 1) * P,
                               e5 * 512: e5 * 512 + 512], stg[:])
        if jt % 4 == 3:
            nc.gpsimd.collective_compute(
                "AllGather", mybir.AluOpType.bypass, replica_groups=groups,
                ins=[v_own[jt // 4][:]], outs=[v_gath[jt // 4][:]])

    for half in range(2):
        for r in range(2):
            o = (r * JTH + half * 4) * D
            dst = vp_s[:, o: o + 4 * D]
            dst = dst.rearrange("p (jtl e) -> p jtl e", jtl=4, e=D)
            src_ap = v_gath[half][r].rearrange("(jtl p) e -> p jtl e", p=P)
            nc.gpsimd.dma_start(dst[:, :, :], src_ap[:, :, :])

    # ---- phase E: QT + q sumsq -------------------------------------------
    # et-outer with the wqt tile stationary across both i5 halves (half the
    # LDWEIGHTS); ones-matmuls lag one e-tile behind the squares
    qsq_pss = [psr_p.tile([1, 512], dtf, tag="psrow", name=f"qsq_ps{i}")
               for i in range(I5)]
    sqs = []
    for et in range(ET):
        pss = [ps_p.tile([P, 512], dtf, tag="psmm", name=f"ps{i}")
               for i in range(I5)]
        for dc in range(DC):
            for i5 in range(I5):
                nc.tensor.matmul(
                    pss[i5][:],
                    lhsT=wqt_s[:, dc * D + et * P: dc * D + (et + 1) * P],
                    rhs=xq_rhs(i5, dc),
                    start=(dc == 0), stop=(dc == DC - 1),
                )
        for i5 in range(I5):
            qtsl = qt_s[:, et * HALF + i5 * 512: et * HALF + i5 * 512 + 512]
            nc.vector.tensor_copy(qtsl, pss[i5][:])
            sq = sq_p.tile([P, 512], dtb, tag="sq")
            nc.scalar.square(sq[:], pss[i5][:])
            sqs.append(sq)
        if et > 0:
            for i5 in range(I5):
                nc.tensor.matmul(qsq_pss[i5][:], lhsT=ones_b[:],
                                 rhs=sqs[2 * (et - 1) + i5][:],
                                 start=(et == 1), stop=False)
    for i5 in range(I5):
        nc.tensor.matmul(qsq_pss[i5][:], lhsT=ones_b[:],
                         rhs=sqs[2 * (ET - 1) + i5][:], start=False, stop=True)
        nc.vector.tensor_copy(qsq_row[0:1, i5 * 512: i5 * 512 + 512],
                              qsq_pss[i5][:])

    # ---- phase F: score blocks, then --------------------------------------
    # ---- phase G: out blocks ---------------------------------------------
    # s-outer with the kt tile stationary across both 512-query blocks
    # (halves the LDWEIGHTS); per-(ib,s) mask kind applied on evict
    ndiag = [0] * I5
    diag_idx = [[None] * JT for _ in range(I5)]
    for ib in range(I5):
        for s in range(JT):
            if _fkind(ib, s) == 'diag':
                diag_idx[ib][s] = ndiag[ib]
                ndiag[ib] += 1
    st_blks = [w_p.tile([P, JT * 512], dtb, tag="w", name=f"st_blk{i}")
               for i in range(I5)]
    for s in range(JT):
        pss = [ps_p.tile([P, 512], dtf, tag="psmm", name=f"ps{i}")
               for i in range(I5)]
        for et in range(ET):
            for ib in range(I5):
                nc.tensor.matmul(
                    pss[ib][:],
                    lhsT=kt_s[:, et * S + s * P: et * S + (s + 1) * P],
                    rhs=qt_s[:, et * HALF + ib * 512: et * HALF + ib * 512 + 512],
                    start=(et == 0), stop=(et == ET - 1),
                )
        for ib in range(I5):
            kind = _fkind(ib, s)
            dst = st_blks[ib][:, s * 512:(s + 1) * 512]
            if kind == 'diag':
                mk = mk_p.tile([P, 512], dt8, tag="mk")
                nc.scalar.dma_start(
                    mk[:], maskd[(ib * 8 + diag_idx[ib][s]) * P:
                                 (ib * 8 + diag_idx[ib][s] + 1) * P, :])
                nc.vector.tensor_mul(dst, pss[ib][:], mk[:])
            elif kind == 'pad':
                nc.vector.tensor_scalar_mul(dst, pss[ib][:], pad_s[:, s:s + 1])
            else:  # 'ones': fully above the diagonal, mask is all-ones
                nc.vector.tensor_copy(dst, pss[ib][:])
    # q-norm chain (deferred so F's matmuls aren't blocked behind it)
    for c in range(ET):
        pc = psc_p.tile([P, 1], dtf, tag="pscol")
        nc.tensor.matmul(pc[:], lhsT=qsq_row[0:1, c * P:(c + 1) * P],
                         rhs=ones_f[:], start=True, stop=True)
        nc.vector.tensor_copy(qsq_col[:, c:c + 1], pc[:])
    nc.vector.reciprocal(qrec_col[:], qsq_col[:])
    nc.scalar.sqrt(qinv_col[:], qrec_col[:])

    # st tile stationary across the two d5 halves (halves the LDWEIGHTS)
    for ib in range(I5):
        st_blk = st_blks[ib]
        for itl in range(4):
            g = ib * 4 + itl          # local i-tile
            pss = [ps_p.tile([P, 512], dtf, tag="psmm", name=f"psg{i}")
                   for i in range(2)]
            for s in range(JT):
                for d5 in range(2):
                    nc.tensor.matmul(
                        pss[d5][:],
                        lhsT=st_blk[:, s * 512 + itl * P: s * 512 + (itl + 1) * P],
                        rhs=vp_s[:, s * D + d5 * 512: s * D + d5 * 512 + 512],
                        start=(s == 0), stop=(s == JT - 1),
                    )
            for d5 in range(2):
                ot = ev_p.tile([P, 512], dtf, tag="ev")
                nc.vector.tensor_scalar_mul(ot[:], pss[d5][:], qinv_col[:, g:g + 1])
                nc.sync.dma_start(out[g * P:(g + 1) * P, d5 * 512: d5 * 512 + 512],
                                  ot[:])


def _build():
    if "nc" in _CACHE:
        return _CACHE["nc"]
    import concourse.tile as tile
    from concourse import bacc, mybir

    dtb = mybir.dt.bfloat16
    dtf = mybir.dt.float32
    nc = bacc.Bacc("TRN2", target_bir_lowering=False, debug=False,
                   enable_asserts=True, num_devices=N_CORES)
    xq = nc.dram_tensor("xq", [P, I5 * DC * 512], dtb, kind="ExternalInput").ap()
    wkt = nc.dram_tensor("wkt", [P, ET * DC * P], dtb, kind="ExternalInput").ap()
    wvt = nc.dram_tensor("wvt", [D, D], dtb, kind="ExternalInput").ap()
    wqt = nc.dram_tensor("wqt", [D, D], dtb, kind="ExternalInput").ap()
    maskd = nc.dram_tensor("maskd", [I5 * 8 * P, 512],
                           mybir.dt.float8e4,
                           kind="ExternalInput").ap()
    padc = nc.dram_tensor("padc", [P, JT], dtf, kind="ExternalInput").ap()
    out = nc.dram_tensor("out", [HALF, D], dtf, kind="ExternalOutput").ap()
    kt_own = [nc.dram_tensor(f"kt_own{i}", [D, 512], dtb).ap() for i in range(2)]
    kt_gath = [nc.dram_tensor(f"kt_gath{i}", [2, D, 512], dtb).ap()
               for i in range(2)]
    v_own = [nc.dram_tensor(f"v_own{i}", [512, D], dtb).ap() for i in range(2)]
    v_gath = [nc.dram_tensor(f"v_gath{i}", [2, 512, D], dtb).ap()
              for i in range(2)]

    from contextlib import ExitStack
    with tile.TileContext(nc) as tc:
        with ExitStack() as ctx:
            _emit(ctx, tc, xq, wkt, wvt, wqt, maskd, padc, out,
                  kt_own, kt_gath, v_own, v_gath)
    nc.compile()
    _CACHE["nc"] = nc
    return nc


def make_in_maps(X, masks, Wq, Wk, Wv):
    """Host-side sharding/layout: one input map per core."""
    in_maps = []
    wkt_h = np.ascontiguousarray(Wk.T).astype(BF16)   # [D, D] = [d, e]
    wvt_h = np.ascontiguousarray(Wv.T).astype(BF16)
    wqt_h = np.ascontiguousarray(Wq.T).astype(BF16)
    # wkt pretiled: img[p, (et*DC+dc)*128 + c] = wkt[dc*128+p, et*128+c]
    wkt_img = np.ascontiguousarray(
        wkt_h.reshape(DC, P, ET, P).transpose(1, 2, 0, 3).reshape(P, ET * DC * P))
    keycols = np.concatenate(
        [np.arange(t * P, (t + 1) * P) for t in KEYTILE])   # slot-order keys
    for c in range(N_CORES):
        b, h = c // 2, c % 2
        XT = X[b].T.astype(BF16)                            # [D, S]
        qcols = np.concatenate(
            [np.arange(t * P, (t + 1) * P) for t in TILES[h]])
        xo = XT[:, qcols]                                   # [D, 1024]
        # xq pretiled: img[p, (j5*DC+dc)*512 + c] = xo[dc*128+p, j5*512+c]
        xq_img = np.ascontiguousarray(
            xo.reshape(DC, P, I5, 512).transpose(1, 2, 0, 3)
            .reshape(P, I5 * DC * 512))
        padbit = (masks[b] == 0)                            # True = padded key
        # near-diagonal mask blocks, in (ib, slot) emission order
        mrows = []
        for ib in range(I5):
            icols = qcols[ib * 512:(ib + 1) * 512]
            for s in range(JT):
                if _fkind(ib, s) != 'diag':
                    continue
                jrows = keycols[s * P:(s + 1) * P]
                m = ((jrows[:, None] > icols[None, :]) |
                     padbit[jrows][:, None]).astype(BF16)
                mrows.append(m)
        maskd_h = np.ascontiguousarray(
            np.concatenate(mrows, axis=0).astype(ml_dtypes.float8_e4m3fn))
        padc = np.ascontiguousarray(
            padbit[keycols].astype(np.float32).reshape(JT, P).T)  # [128, JT]
        in_maps.append({
            "xq": xq_img,
            "wkt": wkt_img,
            "wvt": wvt_h,
            "wqt": wqt_h,
            "maskd": maskd_h,
            "padc": padc,
        })
    return in_maps


def run(in_maps, **kw):
    from concourse.bass_utils import run_bass_kernel_spmd
    nc = _build()
    return run_bass_kernel_spmd(nc, in_maps, list(range(N_CORES)), **kw)


def kernel(X, masks, Wq, Wk, Wv):
    X = np.asarray(X, dtype=np.float32)
    masks = np.asarray(masks)
    res = run(make_in_maps(X, masks, np.asarray(Wq, np.float32),
                           np.asarray(Wk, np.float32), np.asarray(Wv, np.float32)))
    out = np.empty((B, S, D), np.float32)
    for c in range(N_CORES):
        b, h = c // 2, c % 2
        for g, t in enumerate(TILES[h]):
            out[b, t * P:(t + 1) * P, :] = res.results[c]["out"][g * P:(g + 1) * P]
    return out
